# revision 23
# baseline (speedup 1.0000x reference)
"""Trainium2 Bass kernel for nn_AttentionLayer (sparse_attention).

Computes, for inputs lstm_lt (B,L,H), lstm_rt (B,R,H), atten_W (H,D),
diagnoal_W (1,1,D):

    atten_lt = tanh(lstm_lt @ W) * diag
    atten_rt = tanh(lstm_rt @ W)
    out      = softmax(atten_lt @ atten_rt^T, axis=-1)      # (B, L, R)

Strategy: pure data parallel over the batch dim across 8 NeuronCores
(8 batches per core).  The host pre-transposes the lstm tensors to
(B, H, L) and casts to bf16 so the contraction dim (H) lands on SBUF
partitions with no on-device transposes.  Per batch, the projections are
computed directly in transposed layout P^T = W^T @ lstm^T (D on
partitions), which is exactly the layout the scores matmul needs for
both operands.  Softmax skips the max-subtraction (scores are O(1),
exp cannot overflow) and uses the ScalarE activation accum_out to get
row sums for free.
"""

import numpy as np
import ml_dtypes

B, L, R, H, D = 64, 512, 512, 512, 256
N_CORES = 8
KB = B // N_CORES  # batches per core

_CACHE = {}

OUT_BF16 = True  # device writes bf16 probs; host casts to fp32


def _build_program(reps=1):
    import concourse.bass as bass  # noqa: F401
    import concourse.tile as tile
    from concourse import bacc, mybir

    f32 = mybir.dt.float32
    bf16 = mybir.dt.bfloat16
    out_dt = bf16 if OUT_BF16 else f32
    AF = mybir.ActivationFunctionType
    Alu = mybir.AluOpType

    nc = bacc.Bacc(
        "TRN2",
        target_bir_lowering=False,
        debug=False,
        enable_asserts=False,
        num_devices=N_CORES,
    )
    ltT = nc.dram_tensor("ltT", [KB, H, L], bf16, kind="ExternalInput").ap()
    rtT = nc.dram_tensor("rtT", [KB, H, R], bf16, kind="ExternalInput").ap()
    w = nc.dram_tensor("w", [H, D], bf16, kind="ExternalInput").ap()
    diag = nc.dram_tensor("diag", [D, 1], f32, kind="ExternalInput").ap()
    out = nc.dram_tensor("out", [KB, L, R], out_dt, kind="ExternalOutput").ap()

    HT = H // 128  # 4 contraction tiles
    DT = D // 128  # 2 projection-output tiles
    LT = L // 128  # 4 score-output tiles

    with tile.TileContext(nc) as tc:
        with (
            tc.tile_pool(name="const", bufs=1) as cpool,
            tc.tile_pool(name="ins", bufs=3) as inpool,
            tc.tile_pool(name="proj", bufs=3) as ppool,
            tc.tile_pool(name="soft", bufs=3) as spool,
            tc.tile_pool(name="stats", bufs=4) as stpool,
            tc.tile_pool(name="outs", bufs=3) as opool,
            tc.tile_pool(name="ppsum", bufs=2, space="PSUM") as ppsum,
            tc.tile_pool(name="spsum", bufs=2, space="PSUM") as spsum,
        ):
            w_sb = cpool.tile([128, HT, D], bf16)
            nc.sync.dma_start(w_sb[:], w.rearrange("(k p) d -> p k d", p=128))
            diag_sb = cpool.tile([128, DT], f32)
            nc.sync.dma_start(diag_sb[:], diag.rearrange("(t p) o -> p (t o)", p=128))

            # Warm-up while the first loads are in flight: dummy matmuls push
            # the PE HAM past its ~3.4us activity window so real matmuls start
            # at 2.4 GHz, and a dummy tanh pulls the ACT table load (~2.7us)
            # off batch 0's critical path.  Inputs are junk SBUF; the PSUM
            # scratch slot is released before the first real projection needs
            # it (and every real accumulation starts with start=True anyway).
            junk = cpool.tile([128, 512], bf16)
            nc.gpsimd.memset(junk[:], 0.0)
            warm_ps = ppsum.tile([128, DT, L], f32, name="warm_ps", tag="ps")
            for _ in range(8):
                nc.tensor.matmul(
                    warm_ps[:, 0, :], junk[:, 0:128], junk[:], start=True, stop=True
                )
            warm_act = cpool.tile([128, 1], bf16)
            nc.scalar.activation(warm_act[:], junk[:, 0:1], AF.Tanh)

            first = [True]

            def emit_load_proj(b):
                """DMA loads + projection matmuls + tanh + diag for batch b.
                Returns (pld, prt) bf16 tiles [(128, DT, L/R)]."""
                lt_sb = inpool.tile([128, HT, L], bf16, name="lt_sb")
                rt_sb = inpool.tile([128, HT, R], bf16, name="rt_sb")
                nc.scalar.dma_start(
                    lt_sb[:], ltT[b].rearrange("(k p) l -> p k l", p=128)
                )
                nc.sync.dma_start(
                    rt_sb[:], rtT[b].rearrange("(k p) l -> p k l", p=128)
                )

                ps_l = ppsum.tile([128, DT, L], f32, name="ps", tag="ps")
                for dd in range(DT):
                    dsl = slice(dd * 128, (dd + 1) * 128)
                    for k in range(HT):
                        nc.tensor.matmul(
                            ps_l[:, dd, :], w_sb[:, k, dsl], lt_sb[:, k, :],
                            start=(k == 0), stop=(k == HT - 1),
                        )
                plt = ppool.tile([128, DT, L], bf16, name="plt")
                nc.scalar.activation(plt[:], ps_l[:], AF.Tanh)
                pld = ppool.tile([128, DT, L], bf16, name="pld")
                for dd in range(DT):
                    nc.vector.tensor_scalar_mul(
                        pld[:, dd, :], plt[:, dd, :], diag_sb[:, dd : dd + 1]
                    )

                ps_r = ppsum.tile([128, DT, R], f32, name="ps_r", tag="ps")
                for dd in range(DT):
                    dsl = slice(dd * 128, (dd + 1) * 128)
                    for k in range(HT):
                        nc.tensor.matmul(
                            ps_r[:, dd, :], w_sb[:, k, dsl], rt_sb[:, k, :],
                            start=(k == 0), stop=(k == HT - 1),
                        )
                prt = ppool.tile([128, DT, R], bf16, name="prt")
                nc.scalar.activation(prt[:], ps_r[:], AF.Tanh)
                return pld, prt

            def emit_scores_softmax(b, pld, prt, last=False):
                """Scores + softmax + store for batch b, in two L-halves of
                2 PSUM banks each; each half's softmax chain is independent.
                For the final half of the last batch, a per-tile fast path
                (ACT accum_out + per-tile recip/store) shortens the kernel
                tail."""
                sdump = spool.tile([128, R], bf16, name="sdump")
                for h in range(LT // 2):
                    ss = spsum.tile([128, 2, R], f32, name="ss", tag="ss")
                    for ii in range(2):
                        i = 2 * h + ii
                        lsl = slice(i * 128, (i + 1) * 128)
                        for dd in range(DT):
                            nc.tensor.matmul(
                                ss[:, ii, :], pld[:, dd, lsl], prt[:, dd, :],
                                start=(dd == 0), stop=(dd == DT - 1),
                            )
                    if last and h == LT // 2 - 1:
                        for ii in range(2):
                            i = 2 * h + ii
                            e1 = spool.tile([128, R], bf16, name="e1")
                            ssum1 = stpool.tile([128, 1], f32, name="ssum1")
                            nc.scalar.activation(
                                e1[:], ss[:, ii, :], AF.Exp, accum_out=ssum1[:]
                            )
                            rcp1 = stpool.tile([128, 1], f32, name="rcp1")
                            nc.vector.reciprocal(rcp1[:], ssum1[:])
                            o1 = opool.tile([128, R], out_dt, name="o1")
                            nc.vector.tensor_scalar_mul(o1[:], e1[:], rcp1[:])
                            nc.sync.dma_start(
                                out[b, 128 * i : 128 * (i + 1), :], o1[:]
                            )
                        continue
                    e = spool.tile([128, 2, R], bf16, name="e")
                    nc.scalar.activation(e[:], ss[:], AF.Exp)
                    ssum = stpool.tile([128, 2], f32, name="ssum")
                    for ii in range(2):
                        nc.vector.tensor_scalar(
                            sdump[:], e[:, ii, :], 1.0, 0.0,
                            op0=Alu.mult, op1=Alu.add,
                            accum_out=ssum[:, ii : ii + 1],
                        )
                    rcp = stpool.tile([128, 2], f32, name="rcp")
                    nc.vector.reciprocal(rcp[:], ssum[:])
                    o = opool.tile([128, 2, R], out_dt, name="o")
                    for ii in range(2):
                        nc.vector.tensor_scalar_mul(
                            o[:, ii, :], e[:, ii, :], rcp[:, ii : ii + 1]
                        )
                    nc.sync.dma_start(
                        out[b, 256 * h : 256 * (h + 1), :].rearrange(
                            "(i p) r -> p i r", p=128
                        ),
                        o[:],
                    )

            # Two-stage software pipeline: proj(b+1) is emitted before
            # scores(b) so the PE stream never waits on tanh.
            batches = [bb for _ in range(reps) for bb in range(KB)]
            prev = None
            for b in batches:
                cur = (b, *emit_load_proj(b))
                if prev is not None:
                    emit_scores_softmax(*prev)
                prev = cur
            emit_scores_softmax(*prev, last=True)

    nc.compile()
    return nc


def _get_program(reps=1):
    key = ("nc", reps)
    if key not in _CACHE:
        _CACHE[key] = _build_program(reps)
    return _CACHE[key]


def _get_runner(reps=1):
    """Build (once) a jitted shard_map executable over the 8 cores.

    Returns run(in_maps) -> list[dict] of per-core outputs.
    """
    key = ("runner", reps)
    if key in _CACHE:
        return _CACHE[key]

    import jax
    from jax.sharding import Mesh, PartitionSpec
    from jax.experimental.shard_map import shard_map
    import concourse.mybir as mybir
    from concourse.bass2jax import _bass_exec_p, install_neuronx_cc_hook

    nc = _get_program(reps)
    install_neuronx_cc_hook()

    partition_name = nc.partition_id_tensor.name if nc.partition_id_tensor else None
    in_names, out_names, out_avals, zero_outs = [], [], [], []
    for alloc in nc.m.functions[0].allocations:
        if not isinstance(alloc, mybir.MemoryLocationSet):
            continue
        name = alloc.memorylocations[0].name
        if alloc.kind == "ExternalInput":
            if name != partition_name:
                in_names.append(name)
        elif alloc.kind == "ExternalOutput":
            shape = tuple(alloc.tensor_shape)
            dtype = mybir.dt.np(alloc.dtype)
            out_names.append(name)
            out_avals.append(jax.core.ShapedArray(shape, dtype))
            zero_outs.append(np.zeros(shape, dtype))
    n_params = len(in_names)
    all_in_names = list(in_names) + list(out_names)
    if partition_name is not None:
        all_in_names.append(partition_name)

    def _body(*args):
        operands = list(args)
        if partition_name is not None:
            from concourse.bass2jax import partition_id_tensor

            operands.append(partition_id_tensor())
        return tuple(
            _bass_exec_p.bind(
                *operands,
                out_avals=tuple(out_avals),
                in_names=tuple(all_in_names),
                out_names=tuple(out_names),
                lowering_input_output_aliases=(),
                sim_require_finite=True,
                sim_require_nnan=True,
                nc=nc,
            )
        )

    devices = jax.devices()[:N_CORES]
    mesh = Mesh(np.asarray(devices), ("core",))
    in_specs = (PartitionSpec("core"),) * (n_params + len(out_names))
    out_specs = (PartitionSpec("core"),) * len(out_names)
    sharded = jax.jit(
        shard_map(
            _body, mesh=mesh, in_specs=in_specs, out_specs=out_specs, check_rep=False
        ),
        keep_unused=True,
    )
    concat_zeros = [
        np.zeros((N_CORES * z.shape[0], *z.shape[1:]), z.dtype) for z in zero_outs
    ]

    def run(in_maps):
        concat_in = [
            np.concatenate([np.asarray(in_maps[c][nm]) for c in range(N_CORES)], axis=0)
            for nm in in_names
        ]
        outs = sharded(*concat_in, *concat_zeros)
        return [
            {
                nm: np.asarray(outs[i]).reshape(N_CORES, *out_avals[i].shape)[c]
                for i, nm in enumerate(out_names)
            }
            for c in range(N_CORES)
        ]

    _CACHE[key] = run
    return run


def _run(lstm_lt, lstm_rt, atten_W, diagnoal_W, reps=1):
    bf = ml_dtypes.bfloat16
    ltT = np.ascontiguousarray(np.asarray(lstm_lt).astype(bf).transpose(0, 2, 1))
    rtT = np.ascontiguousarray(np.asarray(lstm_rt).astype(bf).transpose(0, 2, 1))
    w = np.ascontiguousarray(np.asarray(atten_W).astype(bf))
    diag = np.ascontiguousarray(
        np.asarray(diagnoal_W).astype(np.float32).reshape(D, 1)
    )

    in_maps = [
        {
            "ltT": ltT[c * KB : (c + 1) * KB],
            "rtT": rtT[c * KB : (c + 1) * KB],
            "w": w,
            "diag": diag,
        }
        for c in range(N_CORES)
    ]
    res = _get_runner(reps)(in_maps)
    out = np.concatenate([res[c]["out"] for c in range(N_CORES)], axis=0)
    return out.astype(np.float32), None


def kernel(lstm_lt, lstm_rt, atten_W, diagnoal_W):
    out, _ = _run(lstm_lt, lstm_rt, atten_W, diagnoal_W)
    return out


# revision 25
# speedup vs baseline: 3.7141x; 3.7141x over previous
"""Trainium2 Bass kernel for nn_AttentionLayer (sparse_attention).

Computes, for inputs lstm_lt (B,L,H), lstm_rt (B,R,H), atten_W (H,D),
diagnoal_W (1,1,D):

    atten_lt = tanh(lstm_lt @ W) * diag
    atten_rt = tanh(lstm_rt @ W)
    out      = softmax(atten_lt @ atten_rt^T, axis=-1)      # (B, L, R)

Strategy: pure data parallel over the batch dim across 8 NeuronCores
(8 batches per core).  The host pre-transposes the lstm tensors to
(B, H, L) and casts to bf16 so the contraction dim (H) lands on SBUF
partitions with no on-device transposes.  Per batch, the projections are
computed directly in transposed layout P^T = W^T @ lstm^T (D on
partitions), which is exactly the layout the scores matmul needs for
both operands.  Softmax skips the max-subtraction (scores are O(1),
exp cannot overflow) and uses the ScalarE activation accum_out to get
row sums for free.
"""

import numpy as np
import ml_dtypes

B, L, R, H, D = 64, 512, 512, 512, 256
N_CORES = 8
KB = B // N_CORES  # batches per core

_CACHE = {}

OUT_BF16 = True  # device writes bf16 probs; host casts to fp32


def _build_program(reps=1):
    import concourse.bass as bass  # noqa: F401
    import concourse.tile as tile
    from concourse import bacc, mybir

    f32 = mybir.dt.float32
    bf16 = mybir.dt.bfloat16
    out_dt = bf16 if OUT_BF16 else f32
    AF = mybir.ActivationFunctionType
    Alu = mybir.AluOpType

    nc = bacc.Bacc(
        "TRN2",
        target_bir_lowering=False,
        debug=False,
        enable_asserts=False,
        num_devices=N_CORES,
    )
    ltT = nc.dram_tensor("ltT", [KB, H, L], bf16, kind="ExternalInput").ap()
    rtT = nc.dram_tensor("rtT", [KB, H, R], bf16, kind="ExternalInput").ap()
    w = nc.dram_tensor("w", [H, D], bf16, kind="ExternalInput").ap()
    diag = nc.dram_tensor("diag", [D, 1], f32, kind="ExternalInput").ap()
    out = nc.dram_tensor("out", [KB, L, R], out_dt, kind="ExternalOutput").ap()

    HT = H // 128  # 4 contraction tiles
    DT = D // 128  # 2 projection-output tiles
    LT = L // 128  # 4 score-output tiles

    with tile.TileContext(nc) as tc:
        with (
            tc.tile_pool(name="const", bufs=1) as cpool,
            tc.tile_pool(name="ins", bufs=3) as inpool,
            tc.tile_pool(name="proj", bufs=3) as ppool,
            tc.tile_pool(name="soft", bufs=3) as spool,
            tc.tile_pool(name="stats", bufs=4) as stpool,
            tc.tile_pool(name="outs", bufs=3) as opool,
            tc.tile_pool(name="ppsum", bufs=2, space="PSUM") as ppsum,
            tc.tile_pool(name="spsum", bufs=2, space="PSUM") as spsum,
        ):
            w_sb = cpool.tile([128, HT, D], bf16)
            nc.sync.dma_start(w_sb[:], w.rearrange("(k p) d -> p k d", p=128))
            diag_sb = cpool.tile([128, DT], f32)
            nc.sync.dma_start(diag_sb[:], diag.rearrange("(t p) o -> p (t o)", p=128))

            # Warm-up while the first loads are in flight: dummy matmuls push
            # the PE HAM past its ~3.4us activity window so real matmuls start
            # at 2.4 GHz, and a dummy tanh pulls the ACT table load (~2.7us)
            # off batch 0's critical path.  Inputs are junk SBUF; the PSUM
            # scratch slot is released before the first real projection needs
            # it (and every real accumulation starts with start=True anyway).
            junk = cpool.tile([128, 512], bf16)
            nc.gpsimd.memset(junk[:], 0.0)
            warm_ps = ppsum.tile([128, DT, L], f32, name="warm_ps", tag="ps")
            for _ in range(8):
                nc.tensor.matmul(
                    warm_ps[:, 0, :], junk[:, 0:128], junk[:], start=True, stop=True
                )
            warm_act = cpool.tile([128, 1], bf16)
            nc.scalar.activation(warm_act[:], junk[:, 0:1], AF.Tanh)

            first = [True]

            def emit_load_proj(b):
                """DMA loads + projection matmuls + tanh + diag for batch b.
                Returns (pld, prt) bf16 tiles [(128, DT, L/R)]."""
                lt_sb = inpool.tile([128, HT, L], bf16, name="lt_sb")
                rt_sb = inpool.tile([128, HT, R], bf16, name="rt_sb")
                nc.scalar.dma_start(
                    lt_sb[:], ltT[b].rearrange("(k p) l -> p k l", p=128)
                )
                nc.sync.dma_start(
                    rt_sb[:], rtT[b].rearrange("(k p) l -> p k l", p=128)
                )

                ps_l = ppsum.tile([128, DT, L], f32, name="ps", tag="ps")
                for dd in range(DT):
                    dsl = slice(dd * 128, (dd + 1) * 128)
                    for k in range(HT):
                        nc.tensor.matmul(
                            ps_l[:, dd, :], w_sb[:, k, dsl], lt_sb[:, k, :],
                            start=(k == 0), stop=(k == HT - 1),
                        )
                plt = ppool.tile([128, DT, L], bf16, name="plt")
                nc.scalar.activation(plt[:], ps_l[:], AF.Tanh)
                pld = ppool.tile([128, DT, L], bf16, name="pld")
                for dd in range(DT):
                    nc.vector.tensor_scalar_mul(
                        pld[:, dd, :], plt[:, dd, :], diag_sb[:, dd : dd + 1]
                    )

                ps_r = ppsum.tile([128, DT, R], f32, name="ps_r", tag="ps")
                for dd in range(DT):
                    dsl = slice(dd * 128, (dd + 1) * 128)
                    for k in range(HT):
                        nc.tensor.matmul(
                            ps_r[:, dd, :], w_sb[:, k, dsl], rt_sb[:, k, :],
                            start=(k == 0), stop=(k == HT - 1),
                        )
                prt = ppool.tile([128, DT, R], bf16, name="prt")
                nc.scalar.activation(prt[:], ps_r[:], AF.Tanh)
                return pld, prt

            def emit_scores_softmax(b, pld, prt, last=False):
                """Scores + softmax + store for batch b, in two L-halves of
                2 PSUM banks each; each half's softmax chain is independent.
                For the final half of the last batch, a per-tile fast path
                (ACT accum_out + per-tile recip/store) shortens the kernel
                tail."""
                sdump = spool.tile([128, R], bf16, name="sdump")
                for h in range(LT // 2):
                    ss = spsum.tile([128, 2, R], f32, name="ss", tag="ss")
                    for ii in range(2):
                        i = 2 * h + ii
                        lsl = slice(i * 128, (i + 1) * 128)
                        for dd in range(DT):
                            nc.tensor.matmul(
                                ss[:, ii, :], pld[:, dd, lsl], prt[:, dd, :],
                                start=(dd == 0), stop=(dd == DT - 1),
                            )
                    if last and h == LT // 2 - 1:
                        for ii in range(2):
                            i = 2 * h + ii
                            e1 = spool.tile([128, R], bf16, name="e1")
                            ssum1 = stpool.tile([128, 1], f32, name="ssum1")
                            nc.scalar.activation(
                                e1[:], ss[:, ii, :], AF.Exp, accum_out=ssum1[:]
                            )
                            rcp1 = stpool.tile([128, 1], f32, name="rcp1")
                            nc.vector.reciprocal(rcp1[:], ssum1[:])
                            o1 = opool.tile([128, R], out_dt, name="o1")
                            nc.vector.tensor_scalar_mul(o1[:], e1[:], rcp1[:])
                            nc.sync.dma_start(
                                out[b, 128 * i : 128 * (i + 1), :], o1[:]
                            )
                        continue
                    e = spool.tile([128, 2, R], bf16, name="e")
                    nc.scalar.activation(e[:], ss[:], AF.Exp)
                    ssum = stpool.tile([128, 2], f32, name="ssum")
                    for ii in range(2):
                        nc.vector.tensor_scalar(
                            sdump[:], e[:, ii, :], 1.0, 0.0,
                            op0=Alu.mult, op1=Alu.add,
                            accum_out=ssum[:, ii : ii + 1],
                        )
                    rcp = stpool.tile([128, 2], f32, name="rcp")
                    nc.vector.reciprocal(rcp[:], ssum[:])
                    o = opool.tile([128, 2, R], out_dt, name="o")
                    for ii in range(2):
                        nc.vector.tensor_scalar_mul(
                            o[:, ii, :], e[:, ii, :], rcp[:, ii : ii + 1]
                        )
                    nc.sync.dma_start(
                        out[b, 256 * h : 256 * (h + 1), :].rearrange(
                            "(i p) r -> p i r", p=128
                        ),
                        o[:],
                    )

            # Two-stage software pipeline: proj(b+1) is emitted before
            # scores(b) so the PE stream never waits on tanh.
            batches = [bb for _ in range(reps) for bb in range(KB)]
            prev = None
            for b in batches:
                cur = (b, *emit_load_proj(b))
                if prev is not None:
                    emit_scores_softmax(*prev)
                prev = cur
            emit_scores_softmax(*prev, last=True)

    nc.compile()
    return nc


def _get_program(reps=1):
    key = ("nc", reps)
    if key not in _CACHE:
        _CACHE[key] = _build_program(reps)
    return _CACHE[key]


def _get_runner(reps=1):
    """Build (once) a jitted shard_map executable over the 8 cores.

    Returns run(in_maps) -> list[dict] of per-core outputs.
    """
    key = ("runner", reps)
    if key in _CACHE:
        return _CACHE[key]

    import jax
    from jax.sharding import Mesh, PartitionSpec
    from jax.experimental.shard_map import shard_map
    import concourse.mybir as mybir
    from concourse.bass2jax import _bass_exec_p, install_neuronx_cc_hook

    nc = _get_program(reps)
    install_neuronx_cc_hook()

    partition_name = nc.partition_id_tensor.name if nc.partition_id_tensor else None
    in_names, out_names, out_avals, zero_outs = [], [], [], []
    for alloc in nc.m.functions[0].allocations:
        if not isinstance(alloc, mybir.MemoryLocationSet):
            continue
        name = alloc.memorylocations[0].name
        if alloc.kind == "ExternalInput":
            if name != partition_name:
                in_names.append(name)
        elif alloc.kind == "ExternalOutput":
            shape = tuple(alloc.tensor_shape)
            dtype = mybir.dt.np(alloc.dtype)
            out_names.append(name)
            out_avals.append(jax.core.ShapedArray(shape, dtype))
            zero_outs.append(np.zeros(shape, dtype))
    n_params = len(in_names)
    all_in_names = list(in_names) + list(out_names)
    if partition_name is not None:
        all_in_names.append(partition_name)

    def _body(*args):
        operands = list(args)
        if partition_name is not None:
            from concourse.bass2jax import partition_id_tensor

            operands.append(partition_id_tensor())
        return tuple(
            _bass_exec_p.bind(
                *operands,
                out_avals=tuple(out_avals),
                in_names=tuple(all_in_names),
                out_names=tuple(out_names),
                lowering_input_output_aliases=(),
                sim_require_finite=True,
                sim_require_nnan=True,
                nc=nc,
            )
        )

    devices = jax.devices()[:N_CORES]
    mesh = Mesh(np.asarray(devices), ("core",))
    in_specs = (PartitionSpec("core"),) * (n_params + len(out_names))
    out_specs = (PartitionSpec("core"),) * len(out_names)
    sharded = jax.jit(
        shard_map(
            _body, mesh=mesh, in_specs=in_specs, out_specs=out_specs, check_rep=False
        ),
        keep_unused=True,
    )
    concat_zeros = [
        np.zeros((N_CORES * z.shape[0], *z.shape[1:]), z.dtype) for z in zero_outs
    ]

    def run(in_maps):
        concat_in = [
            np.concatenate([np.asarray(in_maps[c][nm]) for c in range(N_CORES)], axis=0)
            for nm in in_names
        ]
        outs = sharded(*concat_in, *concat_zeros)
        return [
            {
                nm: np.asarray(outs[i]).reshape(N_CORES, *out_avals[i].shape)[c]
                for i, nm in enumerate(out_names)
            }
            for c in range(N_CORES)
        ]

    _CACHE[key] = run
    return run


def _run(lstm_lt, lstm_rt, atten_W, diagnoal_W, reps=1):
    bf = ml_dtypes.bfloat16
    ltT = np.ascontiguousarray(np.asarray(lstm_lt).astype(bf).transpose(0, 2, 1))
    rtT = np.ascontiguousarray(np.asarray(lstm_rt).astype(bf).transpose(0, 2, 1))
    w = np.ascontiguousarray(np.asarray(atten_W).astype(bf))
    diag = np.ascontiguousarray(
        np.asarray(diagnoal_W).astype(np.float32).reshape(D, 1)
    )

    in_maps = [
        {
            "ltT": ltT[c * KB : (c + 1) * KB],
            "rtT": rtT[c * KB : (c + 1) * KB],
            "w": w,
            "diag": diag,
        }
        for c in range(N_CORES)
    ]
    res = _get_runner(reps)(in_maps)
    out = np.concatenate([res[c]["out"] for c in range(N_CORES)], axis=0)
    return out.astype(np.float32), None


def kernel(lstm_lt, lstm_rt, atten_W, diagnoal_W):
    out, _ = _run(lstm_lt, lstm_rt, atten_W, diagnoal_W)
    return out


# revision 29
# speedup vs baseline: 5.5060x; 1.4825x over previous
"""Trainium2 Bass kernel for nn_AttentionLayer (sparse_attention).

Computes, for inputs lstm_lt (B,L,H), lstm_rt (B,R,H), atten_W (H,D),
diagnoal_W (1,1,D):

    atten_lt = tanh(lstm_lt @ W) * diag
    atten_rt = tanh(lstm_rt @ W)
    out      = softmax(atten_lt @ atten_rt^T, axis=-1)      # (B, L, R)

Strategy: pure data parallel over the batch dim across 8 NeuronCores
(8 batches per core).  The host pre-transposes the lstm tensors to
(B, H, L) and casts to bf16 so the contraction dim (H) lands on SBUF
partitions with no on-device transposes.  Per batch, the projections are
computed directly in transposed layout P^T = W^T @ lstm^T (D on
partitions), which is exactly the layout the scores matmul needs for
both operands.  Softmax skips the max-subtraction (scores are O(1),
exp cannot overflow) and uses the ScalarE activation accum_out to get
row sums for free.
"""

import numpy as np
import ml_dtypes

B, L, R, H, D = 64, 512, 512, 512, 256
N_CORES = 8
KB = B // N_CORES  # batches per core

_CACHE = {}

OUT_BF16 = True  # device writes bf16 probs; host casts to fp32


def _build_program(reps=1):
    import concourse.bass as bass  # noqa: F401
    import concourse.tile as tile
    from concourse import bacc, mybir

    f32 = mybir.dt.float32
    bf16 = mybir.dt.bfloat16
    out_dt = bf16 if OUT_BF16 else f32
    AF = mybir.ActivationFunctionType
    Alu = mybir.AluOpType

    nc = bacc.Bacc(
        "TRN2",
        target_bir_lowering=False,
        debug=False,
        enable_asserts=False,
        num_devices=N_CORES,
    )
    ltT = nc.dram_tensor("ltT", [KB, H, L], bf16, kind="ExternalInput").ap()
    rtT = nc.dram_tensor("rtT", [KB, H, R], bf16, kind="ExternalInput").ap()
    w = nc.dram_tensor("w", [H, D], bf16, kind="ExternalInput").ap()
    diag = nc.dram_tensor("diag", [D, 1], f32, kind="ExternalInput").ap()
    out = nc.dram_tensor("out", [KB, L, R], out_dt, kind="ExternalOutput").ap()

    HT = H // 128  # 4 contraction tiles
    DT = D // 128  # 2 projection-output tiles
    LT = L // 128  # 4 score-output tiles

    with tile.TileContext(nc) as tc:
        with (
            tc.tile_pool(name="const", bufs=1) as cpool,
            tc.tile_pool(name="ins", bufs=4) as inpool,
            tc.tile_pool(name="proj", bufs=3) as ppool,
            tc.tile_pool(name="soft", bufs=4) as spool,
            tc.tile_pool(name="stats", bufs=4) as stpool,
            tc.tile_pool(name="outs", bufs=4) as opool,
            tc.tile_pool(name="ppsum", bufs=2, space="PSUM") as ppsum,
            tc.tile_pool(name="spsum", bufs=2, space="PSUM") as spsum,
        ):
            w_sb = cpool.tile([128, HT, D], bf16)
            nc.sync.dma_start(w_sb[:], w.rearrange("(k p) d -> p k d", p=128))
            diag_sb = cpool.tile([128, DT], f32)
            nc.sync.dma_start(diag_sb[:], diag.rearrange("(t p) o -> p (t o)", p=128))

            # Warm-up while the first loads are in flight: dummy matmuls push
            # the PE HAM past its ~3.4us activity window so real matmuls start
            # at 2.4 GHz, and a dummy tanh pulls the ACT table load (~2.7us)
            # off batch 0's critical path.  Inputs are junk SBUF; the PSUM
            # scratch slot is released before the first real projection needs
            # it (and every real accumulation starts with start=True anyway).
            junk = cpool.tile([128, 512], bf16)
            nc.gpsimd.memset(junk[:], 0.0)
            warm_ps = ppsum.tile([128, DT, L], f32, name="warm_ps", tag="ps")
            for _ in range(8):
                nc.tensor.matmul(
                    warm_ps[:, 0, :], junk[:, 0:128], junk[:], start=True, stop=True
                )
            warm_act = cpool.tile([128, 1], bf16)
            nc.scalar.activation(warm_act[:], junk[:, 0:1], AF.Tanh)

            first = [True]

            def emit_load_proj(b):
                """DMA loads + projection matmuls + tanh + diag for batch b.
                Returns (pld, prt) bf16 tiles [(128, DT, L/R)]."""
                lt_sb = inpool.tile([128, HT, L], bf16, name="lt_sb")
                rt_sb = inpool.tile([128, HT, R], bf16, name="rt_sb")
                nc.scalar.dma_start(
                    lt_sb[:], ltT[b].rearrange("(k p) l -> p k l", p=128)
                )
                nc.sync.dma_start(
                    rt_sb[:], rtT[b].rearrange("(k p) l -> p k l", p=128)
                )

                ps_l = ppsum.tile([128, DT, L], f32, name="ps", tag="ps")
                for dd in range(DT):
                    dsl = slice(dd * 128, (dd + 1) * 128)
                    for k in range(HT):
                        nc.tensor.matmul(
                            ps_l[:, dd, :], w_sb[:, k, dsl], lt_sb[:, k, :],
                            start=(k == 0), stop=(k == HT - 1),
                        )
                plt = ppool.tile([128, DT, L], bf16, name="plt")
                nc.scalar.activation(plt[:], ps_l[:], AF.Tanh)
                pld = ppool.tile([128, DT, L], bf16, name="pld")
                for dd in range(DT):
                    nc.vector.tensor_scalar_mul(
                        pld[:, dd, :], plt[:, dd, :], diag_sb[:, dd : dd + 1]
                    )

                ps_r = ppsum.tile([128, DT, R], f32, name="ps_r", tag="ps")
                for dd in range(DT):
                    dsl = slice(dd * 128, (dd + 1) * 128)
                    for k in range(HT):
                        nc.tensor.matmul(
                            ps_r[:, dd, :], w_sb[:, k, dsl], rt_sb[:, k, :],
                            start=(k == 0), stop=(k == HT - 1),
                        )
                prt = ppool.tile([128, DT, R], bf16, name="prt")
                nc.scalar.activation(prt[:], ps_r[:], AF.Tanh)
                return pld, prt

            def emit_scores_softmax(b, pld, prt, last=False):
                """Scores + softmax + store for batch b, in two L-halves of
                2 PSUM banks each; each half's softmax chain is independent.
                For the final half of the last batch, a per-tile fast path
                (ACT accum_out + per-tile recip/store) shortens the kernel
                tail."""
                sdump = spool.tile([128, R], bf16, name="sdump")
                for h in range(LT // 2):
                    ss = spsum.tile([128, 2, R], f32, name="ss", tag="ss")
                    for ii in range(2):
                        i = 2 * h + ii
                        lsl = slice(i * 128, (i + 1) * 128)
                        for dd in range(DT):
                            nc.tensor.matmul(
                                ss[:, ii, :], pld[:, dd, lsl], prt[:, dd, :],
                                start=(dd == 0), stop=(dd == DT - 1),
                            )
                    if last and h == LT // 2 - 1:
                        for ii in range(2):
                            i = 2 * h + ii
                            e1 = spool.tile([128, R], bf16, name="e1")
                            ssum1 = stpool.tile([128, 1], f32, name="ssum1")
                            nc.scalar.activation(
                                e1[:], ss[:, ii, :], AF.Exp, accum_out=ssum1[:]
                            )
                            rcp1 = stpool.tile([128, 1], f32, name="rcp1")
                            nc.vector.reciprocal(rcp1[:], ssum1[:])
                            o1 = opool.tile([128, R], out_dt, name="o1")
                            nc.vector.tensor_scalar_mul(o1[:], e1[:], rcp1[:])
                            nc.sync.dma_start(
                                out[b, 128 * i : 128 * (i + 1), :], o1[:]
                            )
                        continue
                    e = spool.tile([128, 2, R], bf16, name="e")
                    nc.scalar.activation(e[:], ss[:], AF.Exp)
                    ssum = stpool.tile([128, 2], f32, name="ssum")
                    for ii in range(2):
                        nc.vector.tensor_scalar(
                            sdump[:], e[:, ii, :], 1.0, 0.0,
                            op0=Alu.mult, op1=Alu.add,
                            accum_out=ssum[:, ii : ii + 1],
                        )
                    rcp = stpool.tile([128, 2], f32, name="rcp")
                    nc.vector.reciprocal(rcp[:], ssum[:])
                    o = opool.tile([128, 2, R], out_dt, name="o")
                    for ii in range(2):
                        nc.vector.tensor_scalar_mul(
                            o[:, ii, :], e[:, ii, :], rcp[:, ii : ii + 1]
                        )
                    nc.sync.dma_start(
                        out[b, 256 * h : 256 * (h + 1), :].rearrange(
                            "(i p) r -> p i r", p=128
                        ),
                        o[:],
                    )

            # Two-stage software pipeline: proj(b+1) is emitted before
            # scores(b) so the PE stream never waits on tanh.
            batches = [bb for _ in range(reps) for bb in range(KB)]
            prev = None
            for b in batches:
                cur = (b, *emit_load_proj(b))
                if prev is not None:
                    emit_scores_softmax(*prev)
                prev = cur
            emit_scores_softmax(*prev, last=True)

    nc.compile()
    return nc


def _get_program(reps=1):
    key = ("nc", reps)
    if key not in _CACHE:
        _CACHE[key] = _build_program(reps)
    return _CACHE[key]


def _get_runner(reps=1):
    """Build (once) a jitted shard_map executable over the 8 cores.

    Returns run(in_maps) -> list[dict] of per-core outputs.
    """
    key = ("runner", reps)
    if key in _CACHE:
        return _CACHE[key]

    import jax
    from jax.sharding import Mesh, PartitionSpec
    from jax.experimental.shard_map import shard_map
    import concourse.mybir as mybir
    from concourse.bass2jax import _bass_exec_p, install_neuronx_cc_hook

    nc = _get_program(reps)
    install_neuronx_cc_hook()

    partition_name = nc.partition_id_tensor.name if nc.partition_id_tensor else None
    in_names, out_names, out_avals, zero_outs = [], [], [], []
    for alloc in nc.m.functions[0].allocations:
        if not isinstance(alloc, mybir.MemoryLocationSet):
            continue
        name = alloc.memorylocations[0].name
        if alloc.kind == "ExternalInput":
            if name != partition_name:
                in_names.append(name)
        elif alloc.kind == "ExternalOutput":
            shape = tuple(alloc.tensor_shape)
            dtype = mybir.dt.np(alloc.dtype)
            out_names.append(name)
            out_avals.append(jax.core.ShapedArray(shape, dtype))
            zero_outs.append(np.zeros(shape, dtype))
    n_params = len(in_names)
    all_in_names = list(in_names) + list(out_names)
    if partition_name is not None:
        all_in_names.append(partition_name)

    def _body(*args):
        operands = list(args)
        if partition_name is not None:
            from concourse.bass2jax import partition_id_tensor

            operands.append(partition_id_tensor())
        return tuple(
            _bass_exec_p.bind(
                *operands,
                out_avals=tuple(out_avals),
                in_names=tuple(all_in_names),
                out_names=tuple(out_names),
                lowering_input_output_aliases=(),
                sim_require_finite=True,
                sim_require_nnan=True,
                nc=nc,
            )
        )

    devices = jax.devices()[:N_CORES]
    mesh = Mesh(np.asarray(devices), ("core",))
    in_specs = (PartitionSpec("core"),) * (n_params + len(out_names))
    out_specs = (PartitionSpec("core"),) * len(out_names)
    sharded = jax.jit(
        shard_map(
            _body, mesh=mesh, in_specs=in_specs, out_specs=out_specs, check_rep=False
        ),
        keep_unused=True,
    )
    concat_zeros = [
        np.zeros((N_CORES * z.shape[0], *z.shape[1:]), z.dtype) for z in zero_outs
    ]

    def run(in_maps):
        concat_in = [
            np.concatenate([np.asarray(in_maps[c][nm]) for c in range(N_CORES)], axis=0)
            for nm in in_names
        ]
        outs = sharded(*concat_in, *concat_zeros)
        return [
            {
                nm: np.asarray(outs[i]).reshape(N_CORES, *out_avals[i].shape)[c]
                for i, nm in enumerate(out_names)
            }
            for c in range(N_CORES)
        ]

    _CACHE[key] = run
    return run


def _run(lstm_lt, lstm_rt, atten_W, diagnoal_W, reps=1):
    bf = ml_dtypes.bfloat16
    ltT = np.ascontiguousarray(np.asarray(lstm_lt).astype(bf).transpose(0, 2, 1))
    rtT = np.ascontiguousarray(np.asarray(lstm_rt).astype(bf).transpose(0, 2, 1))
    w = np.ascontiguousarray(np.asarray(atten_W).astype(bf))
    diag = np.ascontiguousarray(
        np.asarray(diagnoal_W).astype(np.float32).reshape(D, 1)
    )

    in_maps = [
        {
            "ltT": ltT[c * KB : (c + 1) * KB],
            "rtT": rtT[c * KB : (c + 1) * KB],
            "w": w,
            "diag": diag,
        }
        for c in range(N_CORES)
    ]
    res = _get_runner(reps)(in_maps)
    out = np.concatenate([res[c]["out"] for c in range(N_CORES)], axis=0)
    return out.astype(np.float32), None


def kernel(lstm_lt, lstm_rt, atten_W, diagnoal_W):
    out, _ = _run(lstm_lt, lstm_rt, atten_W, diagnoal_W)
    return out
